# revision 4
# baseline (speedup 1.0000x reference)
"""YOLOv5 Detect head (conv 1x1 + sigmoid decode) on 8 Trainium2 NeuronCores.

Data-parallel over batch: core i handles batches [2i, 2i+1].

Per (batch, level) the work is h = W @ x  (W [255, C], x [C, ny*nx]) followed
by the YOLO decode.  On device we compute psum[s, o] = sum_c x[c, s] * wT[c, o]
with the *data* as the stationary operand (lhsT = x tile [K=128, M<=128 spatial])
and wT [K=128, 256] as the moving operand, so the matmul output lands directly
in [spatial, output-channel] orientation: output rows (a*ny*nx + s) are then
contiguous DMA writes, no transpose needed anywhere.

Decode on-chip:
  s = sigmoid(h)                                   (ACT, psum -> sbuf)
  xy cols (o in {0,1}):  2*stride*s + (grid-0.5)*stride   (DVE scalar_tensor_tensor)
  wh cols (o in {2,3}):  (s*s) * (4*anchor)               (DVE tensor_tensor x2)
  rest: s
"""

import numpy as np
from contextlib import ExitStack

import concourse.bacc as bacc
import concourse.bass as bass
import concourse.mybir as mybir
import concourse.tile as tile
from concourse.bass_utils import run_bass_kernel_spmd

F32 = mybir.dt.float32
F32R = mybir.dt.float32r
AF = mybir.ActivationFunctionType
OP = mybir.AluOpType

NA, NO = 3, 85
B_TOTAL, N_CORES, B_LOC = 16, 8, 2
RHS_W = NA * NO + 1  # 256: pad 255 -> 256 (fp32r full-rate needs moving dim >= 256)
GRP = 8              # slots (128 spatial rows each) per psum/staging group
ROWS_PER_B = 25200

LEVELS = [
    dict(C=256, nx=80, ny=80, stride=8.0,
         anchors=((10.0, 13.0), (16.0, 30.0), (33.0, 23.0)), base=0),
    dict(C=512, nx=40, ny=40, stride=16.0,
         anchors=((30.0, 61.0), (62.0, 45.0), (59.0, 119.0)), base=19200),
    dict(C=1024, nx=20, ny=20, stride=32.0,
         anchors=((116.0, 90.0), (156.0, 198.0), (373.0, 326.0)), base=24000),
]
for _L in LEVELS:
    _L["S"] = _L["nx"] * _L["ny"]
    _L["KT"] = _L["C"] // 128
    _L["nslots"] = (_L["S"] + 127) // 128


def _groups(S):
    """Yield (slot0, n_slots_in_group, rows_in_last_slot)."""
    full, rem = divmod(S, 128)
    gs = [(t0, min(GRP, full - t0), 128) for t0 in range(0, full, GRP)]
    if rem:
        gs.append((full, 1, rem))
    return gs


def _build_program(has_bias: bool):
    nc = bacc.Bacc("TRN2", target_bir_lowering=False, debug=False,
                   num_devices=N_CORES)

    xs = [nc.dram_tensor(f"x{l}", [B_LOC, L["C"], L["S"]], F32R,
                         kind="ExternalInput") for l, L in enumerate(LEVELS)]
    wts = [nc.dram_tensor(f"wt{l}", [L["C"], RHS_W], F32R,
                          kind="ExternalInput") for l, L in enumerate(LEVELS)]
    gxs = [nc.dram_tensor(f"gx{l}", [128, L["nslots"]], F32,
                          kind="ExternalInput") for l, L in enumerate(LEVELS)]
    gys = [nc.dram_tensor(f"gy{l}", [128, L["nslots"]], F32,
                          kind="ExternalInput") for l, L in enumerate(LEVELS)]
    acs = [nc.dram_tensor(f"ac{l}", [128, NA * 2], F32,
                          kind="ExternalInput") for l, L in enumerate(LEVELS)]
    if has_bias:
        bts = [nc.dram_tensor(f"bt{l}", [1, RHS_W], F32,
                              kind="ExternalInput") for l, L in enumerate(LEVELS)]
    out_t = nc.dram_tensor("out", [B_LOC * ROWS_PER_B, NO], F32,
                           kind="ExternalOutput")

    with tile.TileContext(nc) as tc, ExitStack() as ctx:
        cpool = ctx.enter_context(tc.tile_pool(name="consts", bufs=1))
        xpools = [ctx.enter_context(tc.tile_pool(name=f"x{l}", bufs=2))
                  for l in range(3)]
        ppool = ctx.enter_context(tc.tile_pool(name="ps", bufs=2, space="PSUM"))
        spool = ctx.enter_context(tc.tile_pool(name="st", bufs=3))
        tpool = ctx.enter_context(tc.tile_pool(name="tmp", bufs=3))

        # --- resident constants ---
        wt_tiles, gx_tiles, gy_tiles, ac_tiles, bt_tiles = [], [], [], [], []
        for l, L in enumerate(LEVELS):
            KT = L["KT"]
            wt = cpool.tile([128, KT * RHS_W], F32R, tag=f"wt{l}")
            nc.sync.dma_start(
                wt[:].rearrange("p (k c) -> p k c", c=RHS_W),
                wts[l][:].rearrange("(k p) c -> p k c", p=128))
            wt_tiles.append(wt)
            gx = cpool.tile([128, L["nslots"]], F32, tag=f"gx{l}")
            nc.sync.dma_start(gx[:], gxs[l][:])
            gx_tiles.append(gx)
            gy = cpool.tile([128, L["nslots"]], F32, tag=f"gy{l}")
            nc.sync.dma_start(gy[:], gys[l][:])
            gy_tiles.append(gy)
            ac = cpool.tile([128, NA * 2], F32, tag=f"ac{l}")
            nc.sync.dma_start(ac[:], acs[l][:])
            ac_tiles.append(ac)
            if has_bias:
                bt = cpool.tile([1, RHS_W], F32, tag=f"bt{l}")
                nc.sync.dma_start(bt[:], bts[l][:])
                bt_tiles.append(bt)
        if has_bias:
            ones = cpool.tile([1, 128], F32, tag="ones")
            nc.vector.memset(ones[:], 1.0)

        # --- main loop ---
        for b in range(B_LOC):
            for l, L in enumerate(LEVELS):
                KT, S = L["KT"], L["S"]
                x_v = xs[l][b].rearrange("(k p) s -> p k s", p=128)
                for (t0, G, M) in _groups(S):
                    s0 = t0 * 128
                    width = (G - 1) * 128 + M
                    P = 128 if G > 1 else M

                    xt = xpools[l].tile([128, KT * width], F32R, tag=f"x{l}")
                    xt_v = xt[:].rearrange("p (k s) -> p k s", s=width)
                    nc.sync.dma_start(xt_v, x_v[:, :, s0:s0 + width])

                    ps = ppool.tile([128, GRP * RHS_W], F32, tag="ps")
                    for j in range(G):
                        Mj = 128 if j < G - 1 else M
                        po = ps[0:Mj, j * RHS_W:(j + 1) * RHS_W]
                        for k in range(KT):
                            nc.tensor.matmul(
                                po,
                                lhsT=xt_v[:, k, j * 128:j * 128 + Mj],
                                rhs=wt_tiles[l][:].rearrange(
                                    "p (k c) -> p k c", c=RHS_W)[:, k, :],
                                start=(k == 0),
                                stop=(k == KT - 1 and not has_bias))
                        if has_bias:
                            nc.tensor.matmul(po, lhsT=ones[0:1, 0:Mj],
                                             rhs=bt_tiles[l][0:1, :],
                                             start=False, stop=True)

                    st = spool.tile([128, GRP * RHS_W], F32, tag="st")
                    W = G * RHS_W
                    nc.scalar.activation(st[0:P, 0:W], ps[0:P, 0:W], AF.Sigmoid)

                    # decode
                    stv = st[0:P, 0:W].rearrange("p (g w) -> p g w", w=RHS_W)
                    dat = stv[:, :, 0:NA * NO].rearrange(
                        "p g (a o) -> p g a o", o=NO)
                    xsl = dat[:, :, :, 0]
                    ysl = dat[:, :, :, 1]
                    whs = dat[:, :, :, 2:4]
                    gxb = gx_tiles[l][0:P, t0:t0 + G].unsqueeze(2) \
                        .broadcast_to((P, G, NA))
                    gyb = gy_tiles[l][0:P, t0:t0 + G].unsqueeze(2) \
                        .broadcast_to((P, G, NA))
                    two_sigma = 2.0 * L["stride"]
                    nc.vector.scalar_tensor_tensor(
                        xsl, xsl, two_sigma, gxb, OP.mult, OP.add)
                    nc.vector.scalar_tensor_tensor(
                        ysl, ysl, two_sigma, gyb, OP.mult, OP.add)
                    tmp = tpool.tile([128, GRP * NA * 2], F32, tag="tmp")
                    tv = tmp[0:P, 0:G * NA * 2].rearrange(
                        "p (g a j) -> p g a j", a=NA, j=2)
                    nc.vector.tensor_tensor(tv, whs, whs, OP.mult)
                    acb = ac_tiles[l][0:P, :].rearrange(
                        "p (a j) -> p a j", j=2).unsqueeze(1) \
                        .broadcast_to((P, G, NA, 2))
                    nc.vector.tensor_tensor(whs, tv, acb, OP.mult)

                    # output rows: b*25200 + base + a*S + (t0*128 + g*128 + p)
                    for a in range(NA):
                        sb = stv[:, :, a * NO:(a + 1) * NO]
                        row0 = b * ROWS_PER_B + L["base"] + a * S + s0
                        nrows = width
                        dr = out_t[row0:row0 + nrows, :]
                        if G > 1:
                            dr_v = dr.rearrange("(g p) c -> p g c", p=128)
                        else:
                            dr_v = dr.rearrange("(g p) c -> p g c", p=M)
                        nc.sync.dma_start(dr_v, sb)

    nc.compile()
    return nc


_PROG_CACHE = {}


def _get_program(has_bias: bool):
    if has_bias not in _PROG_CACHE:
        _PROG_CACHE[has_bias] = _build_program(has_bias)
    return _PROG_CACHE[has_bias]


def _host_consts(w0, w1, w2, b0, b1, b2, has_bias):
    """Precompute replicated constant arrays shared by all cores."""
    consts = {}
    ws, bs = (w0, w1, w2), (b0, b1, b2)
    for l, L in enumerate(LEVELS):
        wT = np.zeros((L["C"], RHS_W), dtype=np.float32)
        wT[:, :NA * NO] = ws[l].T
        consts[f"wt{l}"] = wT

        nslots, nx, stride, S = L["nslots"], L["nx"], L["stride"], L["S"]
        s = np.arange(nslots * 128)
        valid = s < S
        gx = np.where(valid, (s % nx - 0.5) * stride, 0.0).astype(np.float32)
        gy = np.where(valid, (s // nx - 0.5) * stride, 0.0).astype(np.float32)
        # gx[p, t] for s = t*128 + p
        consts[f"gx{l}"] = np.ascontiguousarray(
            gx.reshape(nslots, 128).T)
        consts[f"gy{l}"] = np.ascontiguousarray(
            gy.reshape(nslots, 128).T)

        ac = (4.0 * np.asarray(L["anchors"], dtype=np.float32)).reshape(1, -1)
        consts[f"ac{l}"] = np.ascontiguousarray(
            np.broadcast_to(ac, (128, NA * 2)))
        if has_bias:
            bt = np.zeros((1, RHS_W), dtype=np.float32)
            bt[0, :NA * NO] = bs[l]
            consts[f"bt{l}"] = bt
    return consts


def _make_in_maps(inputs):
    x0 = np.asarray(inputs["x0"], dtype=np.float32)
    x1 = np.asarray(inputs["x1"], dtype=np.float32)
    x2 = np.asarray(inputs["x2"], dtype=np.float32)
    w0 = np.asarray(inputs["w0"], dtype=np.float32)
    w1 = np.asarray(inputs["w1"], dtype=np.float32)
    w2 = np.asarray(inputs["w2"], dtype=np.float32)
    b0 = np.asarray(inputs["b0"], dtype=np.float32)
    b1 = np.asarray(inputs["b1"], dtype=np.float32)
    b2 = np.asarray(inputs["b2"], dtype=np.float32)

    has_bias = bool(np.any(b0) or np.any(b1) or np.any(b2))
    consts = _host_consts(w0, w1, w2, b0, b1, b2, has_bias)

    xr = [x0.reshape(B_TOTAL, LEVELS[0]["C"], LEVELS[0]["S"]),
          x1.reshape(B_TOTAL, LEVELS[1]["C"], LEVELS[1]["S"]),
          x2.reshape(B_TOTAL, LEVELS[2]["C"], LEVELS[2]["S"])]

    in_maps = []
    for i in range(N_CORES):
        m = dict(consts)
        for l in range(3):
            m[f"x{l}"] = xr[l][B_LOC * i:B_LOC * (i + 1)]
        in_maps.append(m)
    return in_maps, has_bias


def _assemble(results):
    out = np.stack([results[i]["out"].reshape(B_LOC, ROWS_PER_B, NO)
                    for i in range(N_CORES)])
    return np.ascontiguousarray(out.reshape(B_TOTAL, ROWS_PER_B, NO))


def _run(inputs, trace=False):
    in_maps, has_bias = _make_in_maps(inputs)
    nc = _get_program(has_bias)
    res = run_bass_kernel_spmd(nc, in_maps, core_ids=list(range(N_CORES)),
                               trace=trace)
    return _assemble(res.results), res


def kernel(**inputs):
    out, _ = _run(inputs, trace=False)
    return out
